# revision 6
# baseline (speedup 1.0000x reference)
"""Single-head attention (B=4, S=2048, D=E=1024) on 8 TRN2 NeuronCores.

Sharding: data-parallel over (batch, query-half) -> 8 shards. Core c handles
batch b = c//2, query rows h*1024:(h+1)*1024 with h = c%2. Each core
computes the full K/V projection for its batch (duplicated across the pair),
the Q projection for its query half, scores, softmax and the output.

All on-chip compute is done in a "transposed" layout so that every matmul
operand loads naturally (contraction dim on SBUF partitions):
  - host pre-transposes q/k/v to [D, S] and casts to bf16
  - projections produce Q^T [E, SQ], K^T [E, SK] and V [SK, E]
  - scores are computed as S^T = (K^T)^T-contracted [SK, SQ]
  - softmax over SK (partition-tiled) uses exp (no max subtraction: scores
    have std ~1/3, |max| < ~2.5, so exp is safe in fp32/bf16) and a
    ones-vector matmul for the denominator
  - output is produced as O^T [E, SQ]; host transposes back
"""

import sys

if "/opt/trn_rl_repo" not in sys.path:
    sys.path.insert(0, "/opt/trn_rl_repo")

import numpy as np
import ml_dtypes

P = 128
B, S, D, E = 4, 2048, 1024, 1024
SQ = 1024          # query rows per core
SK = 2048          # key/value rows per core (full batch)
DO = D // P        # 8
EO = E // P        # 8
SKT = SK // P      # 16
FD = 512           # matmul moving free dim
NQC = SQ // FD     # 2
NKC = SK // FD     # 4
SCALE = 1.0 / np.sqrt(np.float32(E))

_NC_CACHE = {}


def build_nc(loop_n=None):
    """Build the per-core program. loop_n wraps the whole body in a hardware
    For_i loop (benchmarking only: amortizes dispatch overhead)."""
    import concourse.bacc as bacc
    import concourse.mybir as mybir
    import concourse.tile as tile
    from concourse.bass import ts
    from contextlib import nullcontext

    bf16 = mybir.dt.bfloat16
    f32 = mybir.dt.float32
    Exp = mybir.ActivationFunctionType.Exp
    mult = mybir.AluOpType.mult

    nc = bacc.Bacc("TRN2", target_bir_lowering=False, debug=False)

    qT = nc.dram_tensor("qT", [D, SQ], bf16, kind="ExternalInput").ap()
    kT = nc.dram_tensor("kT", [D, SK], bf16, kind="ExternalInput").ap()
    vT = nc.dram_tensor("vT", [D, SK], bf16, kind="ExternalInput").ap()
    wq = nc.dram_tensor("wq", [D, E], bf16, kind="ExternalInput").ap()
    wk = nc.dram_tensor("wk", [D, E], bf16, kind="ExternalInput").ap()
    wv = nc.dram_tensor("wv", [D, E], bf16, kind="ExternalInput").ap()
    outT = nc.dram_tensor("outT", [E, SQ], f32, kind="ExternalOutput").ap()

    # DRAM [ (o p), s ] viewed as SBUF-tileable [p, o, s]
    qT3 = qT.rearrange("(o p) s -> p o s", p=P)
    kT3 = kT.rearrange("(o p) s -> p o s", p=P)
    vT3 = vT.rearrange("(o p) s -> p o s", p=P)
    wq3 = wq.rearrange("(o p) e -> p o e", p=P)
    wk3 = wk.rearrange("(o p) e -> p o e", p=P)
    wv3 = wv.rearrange("(o p) e -> p o e", p=P)

    with tile.TileContext(nc) as tc:
        with tc.tile_pool(name="persist", bufs=1) as persist, \
             tc.tile_pool(name="wpool", bufs=2) as wpool, \
             tc.tile_pool(name="stream", bufs=3) as stream, \
             tc.tile_pool(name="misc", bufs=1) as misc, \
             tc.tile_pool(name="ostage", bufs=3) as ostage, \
             tc.tile_pool(name="dram", bufs=1, space="DRAM") as dram, \
             tc.tile_pool(name="psum", bufs=4, space="PSUM") as psum, \
             (tc.For_i(0, loop_n, 1) if loop_n else nullcontext()):

            # ---- persistent on-chip tensors -------------------------------
            V_s = persist.tile([P, SKT, E], bf16, tag="V")     # V[sk, e]
            KT_s = persist.tile([P, EO, SK], bf16, tag="KT")   # K^T[e, sk]
            QT_s = persist.tile([P, EO, SQ], bf16, tag="QT")   # Q^T[e, sq]
            E_s = persist.tile([P, SKT, SQ], bf16, tag="EW")   # exp(S^T)[sk, sq]

            ones = misc.tile([P, 1], bf16, tag="ones")
            nc.any.memset(ones[:], 1.0)

            # ---- V = v @ Wv, produced natural [sk, e] ---------------------
            wv_s = wpool.tile([P, DO, E], bf16, tag="w")
            nc.sync.dma_start(wv_s[:], wv3)
            for skt in range(SKT):
                vt = stream.tile([P, DO, P], bf16, tag="xtv")
                nc.sync.dma_start(vt[:], vT3[:, :, ts(skt, P)])
                for c in range(E // FD):
                    ps = psum.tile([P, FD], f32, tag="mm")
                    for do in range(DO):
                        nc.tensor.matmul(
                            ps[:], vt[:, do, :], wv_s[:, do, ts(c, FD)],
                            start=(do == 0), stop=(do == DO - 1),
                        )
                    nc.vector.tensor_copy(V_s[:, skt, ts(c, FD)], ps[:])

            # ---- K^T = (k @ Wk)^T, produced [e, sk] -----------------------
            wk_s = wpool.tile([P, DO, E], bf16, tag="w")
            nc.sync.dma_start(wk_s[:], wk3)
            for c in range(NKC):
                kc = stream.tile([P, DO, FD], bf16, tag="xtc")
                nc.sync.dma_start(kc[:], kT3[:, :, ts(c, FD)])
                for et in range(EO):
                    ps = psum.tile([P, FD], f32, tag="mm")
                    for do in range(DO):
                        nc.tensor.matmul(
                            ps[:], wk_s[:, do, ts(et, P)], kc[:, do, :],
                            start=(do == 0), stop=(do == DO - 1),
                        )
                    nc.vector.tensor_copy(KT_s[:, et, ts(c, FD)], ps[:])

            # ---- Q^T = (q @ Wq)^T, produced [e, sq] -----------------------
            wq_s = wpool.tile([P, DO, E], bf16, tag="w")
            nc.sync.dma_start(wq_s[:], wq3)
            for c in range(NQC):
                qc = stream.tile([P, DO, FD], bf16, tag="xtc")
                nc.sync.dma_start(qc[:], qT3[:, :, ts(c, FD)])
                for et in range(EO):
                    ps = psum.tile([P, FD], f32, tag="mm")
                    for do in range(DO):
                        nc.tensor.matmul(
                            ps[:], wq_s[:, do, ts(et, P)], qc[:, do, :],
                            start=(do == 0), stop=(do == DO - 1),
                        )
                    nc.vector.tensor_copy(QT_s[:, et, ts(c, FD)], ps[:])

            # ---- E = exp(scale * S^T),  S^T[sk, sq] = K Q^T ---------------
            for skt in range(SKT):
                for c in range(NQC):
                    ps = psum.tile([P, FD], f32, tag="mm")
                    for et in range(EO):
                        nc.tensor.matmul(
                            ps[:], KT_s[:, et, ts(skt, P)], QT_s[:, et, ts(c, FD)],
                            start=(et == 0), stop=(et == EO - 1),
                        )
                    nc.scalar.activation(
                        E_s[:, skt, ts(c, FD)], ps[:], Exp, scale=float(SCALE)
                    )

            # ---- softmax denominator: den[sq] = sum_sk E[sk, sq] ----------
            den = misc.tile([1, SQ], f32, tag="den")
            for c in range(NQC):
                psd = psum.tile([1, FD], f32, tag="den", bufs=2)
                for skt in range(SKT):
                    nc.tensor.matmul(
                        psd[:], ones[:, :], E_s[:, skt, ts(c, FD)],
                        start=(skt == 0), stop=(skt == SKT - 1),
                    )
                nc.vector.tensor_copy(den[:, ts(c, FD)], psd[:])
            den_d = dram.tile([1, SQ], f32)
            nc.sync.dma_start(den_d[:], den[:])
            rden = misc.tile([P, SQ], f32, tag="rden")
            nc.sync.dma_start(rden[:], den_d[:].to_broadcast((P, SQ)))
            nc.vector.reciprocal(rden[:], rden[:])

            # ---- O^T[e, sq] = V^T E, then normalize and store -------------
            for et in range(EO):
                for c in range(NQC):
                    ps = psum.tile([P, FD], f32, tag="mm")
                    for skt in range(SKT):
                        nc.tensor.matmul(
                            ps[:], V_s[:, skt, ts(et, P)], E_s[:, skt, ts(c, FD)],
                            start=(skt == 0), stop=(skt == SKT - 1),
                        )
                    ot = ostage.tile([P, FD], f32, tag="ot")
                    nc.vector.tensor_tensor(
                        ot[:], ps[:], rden[:, ts(c, FD)], mult
                    )
                    nc.sync.dma_start(outT[ts(et, P), ts(c, FD)], ot[:])

    nc.compile()
    return nc


def get_nc():
    if "nc" not in _NC_CACHE:
        _NC_CACHE["nc"] = build_nc()
    return _NC_CACHE["nc"]


def make_in_maps(q, k, v, W_q, W_k, W_v):
    bf = ml_dtypes.bfloat16
    wq = np.ascontiguousarray(W_q.astype(bf))
    wk = np.ascontiguousarray(W_k.astype(bf))
    wv = np.ascontiguousarray(W_v.astype(bf))
    kTb = [np.ascontiguousarray(k[b].astype(bf).T) for b in range(B)]
    vTb = [np.ascontiguousarray(v[b].astype(bf).T) for b in range(B)]
    in_maps = []
    for c in range(8):
        b, h = c // 2, c % 2
        qTc = np.ascontiguousarray(q[b, h * SQ:(h + 1) * SQ, :].astype(bf).T)
        in_maps.append({
            "qT": qTc, "kT": kTb[b], "vT": vTb[b],
            "wq": wq, "wk": wk, "wv": wv,
        })
    return in_maps


def kernel(q, k, v, W_q, W_k, W_v):
    from concourse import bass_utils

    nc = get_nc()
    in_maps = make_in_maps(q, k, v, W_q, W_k, W_v)
    res = bass_utils.run_bass_kernel_spmd(nc, in_maps, core_ids=list(range(8)))
    out = np.empty((B, S, E), dtype=np.float32)
    for c in range(8):
        b, h = c // 2, c % 2
        out[b, h * SQ:(h + 1) * SQ, :] = res.results[c]["outT"].T
    return out
